# revision 1
# baseline (speedup 1.0000x reference)
"""MoE layer (E=8 experts, top-2, swiGLU) on 8 TRN2 NeuronCores.

Strategy: expert parallelism. The router (x @ Wr -> top-2 -> softmax gates)
is tiny (<0.1% of FLOPs) and is computed on host to build the dispatch:
tokens are gathered per expert into a padded capacity-C batch, one expert
per core. Each core runs the expert MLP

    y = (silu(X @ W1a + b1a) * (X @ W1b + b1b)) @ W2 + b2, scaled by gate

entirely on device with float32r matmuls (full-rate PE, ~fp32 accuracy).
The host scatter-adds the 8 per-expert outputs back (each token appears in
exactly 2 expert batches) — the EP "combine" step.

Device kernel structure (per core, SPMD — identical program, per-core data):
  - xt  [P, KO1, C]   token block, transposed (D on partitions)   (resident)
  - w2  [P, KO2, D]   expert W2                                    (resident)
  - hT  [P, MP, TB]   swiGLU output, transposed (H on partitions) (per block)
  - W1 streamed from HBM in [P, 2, KO1, 128] column tiles
  GEMM1: H1T[h, t] = sum_k W1[k, h] * X[t, k]   (stationary=W1 tile, moving=xt)
  GEMM2: Y[t, d]   = sum_h hT[h, t] * W2[h, d]  (stationary=hT tile, moving=w2)
  Token dim processed in blocks of TB=512 so hT fits in SBUF; W1 is
  re-streamed once per block.
"""

import math

import numpy as np

import concourse.bacc as bacc
import concourse.bass as bass  # noqa: F401
import concourse.mybir as mybir
import concourse.tile as tile
from concourse.bass_utils import run_bass_kernel_spmd
from concourse.tile import add_dep_helper

P = 128
NCORES = 8

f32 = mybir.dt.float32
f32r = mybir.dt.float32r
SIGMOID = mybir.ActivationFunctionType.Sigmoid
ADD = mybir.AluOpType.add


def _blocks(C, TB):
    """Split C (multiple of 128) into token blocks of at most TB.
    Any remainder block goes FIRST: the small block's swiGLU-latency
    stalls then overlap the DMA-warmup phase instead of the tail."""
    rem = C % TB
    out = []
    t0 = 0
    if rem:
        out.append((0, rem))
        t0 = rem
    while t0 < C:
        out.append((t0, TB))
        t0 += TB
    return out


def _chunks(tw):
    """Split a token block into matmul moving-dim chunks (<=512)."""
    out = []
    c0 = 0
    while c0 < tw:
        out.append((c0, min(512, tw - c0)))
        c0 += 512
    return out


def build_moe_expert_nc(D, H, C, TB=768, has_b1=False, has_b2=False):
    """Build the SPMD per-expert kernel. D % 128 == 0, H % 128 == 0,
    C % 128 == 0 required."""
    KO1 = D // P       # k tiles of GEMM1 (contraction over D)
    MP = H // P        # hidden tiles (per swiGLU half)
    KO2 = H // P       # k tiles of GEMM2 (contraction over H)
    n2chunks = _chunks(D)  # GEMM2 free-dim chunks over D

    nc = bacc.Bacc(None)
    xt_d = nc.declare_dram_parameter("xt", [P, KO1, C], f32r, isOutput=False)
    w1_d = nc.declare_dram_parameter("w1", [MP, P, 2, KO1, P], f32r, isOutput=False)
    w2_d = nc.declare_dram_parameter("w2", [P, KO2, D], f32r, isOutput=False)
    g_d = nc.declare_dram_parameter("g", [P, C // P], f32, isOutput=False)
    if has_b1:
        b1_d = nc.declare_dram_parameter("b1", [P, 2, MP], f32, isOutput=False)
    if has_b2:
        b2_d = nc.declare_dram_parameter("b2", [P, D], f32, isOutput=False)
    y_d = nc.declare_dram_parameter("y", [C, D], f32, isOutput=True)

    blocks = _blocks(C, TB)

    with tile.TileContext(nc) as tc:
        with (
            tc.tile_pool(name="const", bufs=1) as const,
            tc.tile_pool(name="xtp", bufs=2) as xtp,
            tc.tile_pool(name="w1p", bufs=3) as w1p,
            tc.tile_pool(name="ev", bufs=2) as ev,
            tc.tile_pool(name="ps1", bufs=1, space="PSUM") as ps1,
            tc.tile_pool(name="ps2", bufs=2, space="PSUM") as ps2,
        ):
            # Only block 0's tokens load on the sync (HWDGE) queues before
            # compute: HWDGE waits are cumulative per queue, so anything
            # enqueued ahead of the first W1 tile would stall the first
            # matmul. The remaining bulk loads (xt blocks 1+, w2, gates,
            # biases) go on gpsimd (SWDGE) queues with explicit dep edges
            # so they stream DURING compute instead of starving the W1
            # prefetch at t=0.
            # xt blocks share one double-buffered tag: block i+1's tokens
            # stream in (during block i's GEMM1) over the buffer freed when
            # block i-1's GEMM1 retired.
            xt_tiles = [
                xtp.tile([P, KO1, tw], f32r, name=f"xt{bi}", tag="xt")
                if not (bi == 0 and tw > 512)
                else None
                for bi, (t0, tw) in enumerate(blocks)
            ]
            g_sb = const.tile([P, C // P], f32)
            w2_sb = const.tile([P, KO2, D], f32r)
            if has_b1:
                # tiny; read by the very first swiGLU, so load up front
                b1_sb = const.tile([P, 2, MP], f32)
                nc.sync.dma_start(b1_sb[:], b1_d[:])
            if has_b2:
                b2_sb = const.tile([P, D], f32)
                nc.sync.dma_start(b2_sb[:], b2_d[:])
            tw0 = blocks[0][1]
            if xt_tiles[0] is None:
                # split block 0: only chunk 0 (cols 0:512) gates the first
                # matmul on the sync queue; the rest rides SWDGE in parallel
                # and is not needed until the second chunk's matmuls.
                xt0a = xtp.tile([P, KO1, 512], f32r, tag="xt")
                xt0b = xtp.tile([P, KO1, tw0 - 512], f32r, tag="xt")
                nc.sync.dma_start(xt0a[:], xt_d[:, :, :512])
                nc.gpsimd.dma_start(xt0b[:], xt_d[:, :, 512:tw0])
            else:
                xt0a, xt0b = xt_tiles[0], None
                nc.sync.dma_start(xt0a[:], xt_d[:, :, :tw0])

            def xt_rhs(bi, k, c0, cw):
                if bi == 0:
                    if c0 < 512:
                        return xt0a[:, k, c0 : c0 + cw]
                    return xt0b[:, k, c0 - 512 : c0 - 512 + cw]
                return xt_tiles[bi][:, k, c0 : c0 + cw]

            # PE warmup: ~4us of throwaway matmuls on a zeroed scratch tile
            # while the first real loads stream in, so the HAM clock gate is
            # already at 2.4 GHz when real matmuls start.
            warm = const.tile([P, 640], f32r)
            nc.gpsimd.memset(warm[:].bitcast(f32), 0.0)
            for wi in range(13):
                warm_ps = ps1.tile([P, 512], f32, tag="g1_2", name=f"warm_ps{wi}")
                nc.tensor.matmul(
                    warm_ps[:],
                    lhsT=warm[:, :128],
                    rhs=warm[:, 128:640],
                    start=True,
                    stop=True,
                )

            # filled during the main loop: first matmul of (block, mp)
            block_mm = {}

            def _stagger(dma_bi, anchor):
                if anchor is not None:
                    add_dep_helper(
                        dma_bi.ins, anchor.ins, sync=True,
                        reason="stagger bulk DMA behind compute",
                    )

            for bi, (t0, tw) in enumerate(blocks):
                hT = ev.tile([P, MP, tw], f32r, tag="hT", bufs=1)
                # ---- GEMM1 + swiGLU: hT[:, mp, :] for all hidden tiles ----
                for mp in range(MP):
                    w1t = w1p.tile([P, 2, KO1, P], f32r, tag="w1t")
                    nc.sync.dma_start(w1t[:], w1_d[mp])
                    # 3-tag PSUM rotation: reuse distance 1.5 mp-pairs, so
                    # the ACT->DVE->DVE swiGLU chain latency never stalls
                    # the next mp's matmul group.
                    psa = ps1.tile([P, tw], f32, tag=f"g1_{(2 * mp) % 3}")
                    psb = ps1.tile([P, tw], f32, tag=f"g1_{(2 * mp + 1) % 3}")
                    for c0, cw in _chunks(tw):
                        for k in range(KO1):
                            mm = nc.tensor.matmul(
                                psa[:, c0 : c0 + cw],
                                lhsT=w1t[:, 0, k, :],
                                rhs=xt_rhs(bi, k, c0, cw),
                                start=(k == 0),
                                stop=(k == KO1 - 1),
                            )
                            block_mm.setdefault((bi, mp), mm)
                        for k in range(KO1):
                            nc.tensor.matmul(
                                psb[:, c0 : c0 + cw],
                                lhsT=w1t[:, 1, k, :],
                                rhs=xt_rhs(bi, k, c0, cw),
                                start=(k == 0),
                                stop=(k == KO1 - 1),
                            )
                    # silu(a) = a * sigmoid(a); a = psa (+ b1a), b = psb (+ b1b)
                    sg = ev.tile([P, tw], f32, tag="sg")
                    if has_b1:
                        av = ev.tile([P, tw], f32, tag="av")
                        nc.vector.tensor_scalar_add(
                            av[:], psa[:], b1_sb[:, 0, mp : mp + 1]
                        )
                        nc.scalar.activation(sg[:], av[:], SIGMOID)
                        nc.vector.tensor_mul(sg[:], sg[:], av[:])
                        bs = ev.tile([P, tw], f32, tag="bs")
                        nc.vector.tensor_scalar_add(
                            bs[:], psb[:], b1_sb[:, 1, mp : mp + 1]
                        )
                        nc.vector.tensor_mul(hT[:, mp, :], sg[:], bs[:])
                    else:
                        nc.scalar.activation(sg[:], psa[:], SIGMOID)
                        nc.vector.tensor_mul(sg[:], sg[:], psa[:])
                        nc.vector.tensor_mul(hT[:, mp, :], sg[:], psb[:])

                # ---- staggered bulk loads: emitted BEFORE their readers
                # (Tile deps are emission-ordered) but dep-anchored on this
                # block's first matmul so they stream during compute instead
                # of starving the W1/xt0 critical path at t=0.
                if bi == 0:
                    # w2 in 4 chunks spread across block 0's GEMM1 so the
                    # SWDGE bursts never starve the W1 stream for long
                    nw2 = 4
                    kstep = max(1, KO2 // nw2)
                    for ci, k0 in enumerate(range(0, KO2, kstep)):
                        k1 = min(KO2, k0 + kstep)
                        dma = nc.gpsimd.dma_start(
                            w2_sb[:, k0:k1, :], w2_d[:, k0:k1, :]
                        )
                        anchor_mp = min(5 + 3 * ci, MP - 1)
                        _stagger(dma, block_mm.get((0, anchor_mp)))
                    dma = nc.gpsimd.dma_start(g_sb[:], g_d[:])
                    _stagger(dma, block_mm.get((0, 0)))
                if bi + 1 < len(blocks):
                    # next block's tokens stream during THIS block's GEMM2
                    # (a window with no W1 demand)
                    u0, uw = blocks[bi + 1]
                    dma = nc.gpsimd.dma_start(
                        xt_tiles[bi + 1][:], xt_d[:, :, u0 : u0 + uw]
                    )
                    _stagger(dma, block_mm.get((bi, MP - 1)))

                # ---- GEMM2 + gate scale: y rows for this token block ----
                for mt in range(tw // P):
                    ti = t0 // P + mt
                    rows = slice(t0 + mt * P, t0 + (mt + 1) * P)
                    for n0, nw in n2chunks:
                        psy = ps2.tile([P, max(nw, 1)], f32, tag="psy")
                        for k in range(KO2):
                            nc.tensor.matmul(
                                psy[:, :nw],
                                lhsT=hT[:, k, mt * P : (mt + 1) * P],
                                rhs=w2_sb[:, k, n0 : n0 + nw],
                                start=(k == 0),
                                stop=(k == KO2 - 1),
                            )
                        ysb = ev.tile([P, nw], f32, tag="ysb")
                        if has_b2:
                            nc.vector.tensor_tensor(
                                ysb[:], psy[:, :nw], b2_sb[:, n0 : n0 + nw], ADD
                            )
                            nc.vector.tensor_scalar_mul(
                                ysb[:], ysb[:], g_sb[:, ti : ti + 1]
                            )
                        else:
                            nc.vector.tensor_scalar_mul(
                                ysb[:], psy[:, :nw], g_sb[:, ti : ti + 1]
                            )
                        nc.sync.dma_start(y_d[rows, n0 : n0 + nw], ysb[:])
    # run_bass_via_pjrt (the axon execute path) takes a prebuilt module and
    # never finalizes it; Bacc defers register allocation to finalize().
    nc.finalize()
    return nc


def _route(x2, Wr):
    """Top-2 router, numpy fp32 (mirrors jax.lax.top_k + softmax)."""
    n = x2.shape[0]
    ar = np.arange(n)
    z = x2 @ Wr  # [N, E] fp32
    idx1 = z.argmax(axis=1)
    v1 = z[ar, idx1]
    z2 = z.copy()
    z2[ar, idx1] = -np.inf
    idx2 = z2.argmax(axis=1)
    v2 = z2[ar, idx2]
    m = np.maximum(v1, v2)
    e1 = np.exp(v1 - m)
    e2 = np.exp(v2 - m)
    s = e1 + e2
    return idx1, idx2, (e1 / s).astype(np.float32), (e2 / s).astype(np.float32)


def kernel(x, Wr, W1, b1, W2, b2):
    x = np.asarray(x, dtype=np.float32)
    Wr = np.asarray(Wr, dtype=np.float32)
    W1 = np.asarray(W1, dtype=np.float32)
    b1 = np.asarray(b1, dtype=np.float32)
    W2 = np.asarray(W2, dtype=np.float32)
    b2 = np.asarray(b2, dtype=np.float32)

    Bb, T, D = x.shape
    E, _, H2 = W1.shape
    H = H2 // 2
    N = Bb * T
    assert E == NCORES

    x2 = x.reshape(N, D)
    idx1, idx2, g1, g2 = _route(x2, Wr)

    tok = np.concatenate([np.arange(N), np.arange(N)])
    exp = np.concatenate([idx1, idx2])
    gat = np.concatenate([g1, g2])

    toks_e = [tok[exp == e] for e in range(E)]
    gats_e = [gat[exp == e] for e in range(E)]
    counts = np.array([len(t) for t in toks_e])
    C = max(512, int(math.ceil(counts.max() / P) * P))

    has_b1 = bool(np.any(b1))
    has_b2 = bool(np.any(b2))

    nc = build_moe_expert_nc(D, H, C, TB=768, has_b1=has_b1, has_b2=has_b2)

    KO1 = D // P
    MP = H // P
    KO2 = H // P

    in_maps = []
    for e in range(E):
        ce = len(toks_e[e])
        xt = np.zeros((D, C), dtype=np.float32)
        xt[:, :ce] = x2[toks_e[e]].T
        xt_t = np.ascontiguousarray(xt.reshape(KO1, P, C).transpose(1, 0, 2))

        w1_t = np.ascontiguousarray(
            W1[e].reshape(KO1, P, 2, MP, P).transpose(3, 1, 2, 0, 4)
        )
        w2_t = np.ascontiguousarray(W2[e].reshape(KO2, P, D).transpose(1, 0, 2))

        g = np.zeros(C, dtype=np.float32)
        g[:ce] = gats_e[e]
        g_t = np.ascontiguousarray(g.reshape(C // P, P).T)

        im = {"xt": xt_t, "w1": w1_t, "w2": w2_t, "g": g_t}
        if has_b1:
            im["b1"] = np.ascontiguousarray(
                b1[e].reshape(2, MP, P).transpose(2, 0, 1)
            )
        if has_b2:
            im["b2"] = np.ascontiguousarray(np.broadcast_to(b2[e], (P, D)))
        in_maps.append(im)

    res = run_bass_kernel_spmd(nc, in_maps, list(range(NCORES)))

    out = np.zeros((N, D), dtype=np.float32)
    for e in range(E):
        ce = len(toks_e[e])
        out[toks_e[e]] += res.results[e]["y"][:ce]
    return out.reshape(Bb, T, D)



# revision 9
# speedup vs baseline: 1.1462x; 1.1462x over previous
"""MoE layer (E=8 experts, top-2, swiGLU) on 8 TRN2 NeuronCores.

Strategy: balanced expert-block dispatch. The router runs on host; each
core is assigned a fixed pattern of token blocks (same block sizes on
every core -> one SPMD program), and each block is bound to ONE expert
whose weights are streamed per block from per-core DRAM data. Packing
experts into the 8x[4,4,4,5]-tile block grid balances the padded token
count to C = ceil(sum_e ceil(count_e/128) / 8)*128 per core instead of
max_e count (2176 vs 2304 for the balanced-random router here).

All matmul operands are bf16 (PE full rate, FWL weight loads, half the
DMA bytes); accumulation stays fp32 in PSUM, swiGLU runs fp32 on
ACT/DVE, hT is stored bf16, outputs are fp32. Measured bf16 end-to-end
error ~4e-3 of absmax (gate 2e-2).

Device kernel per block bi (TW tokens):
  GEMM1: hT[h, t] = swiGLU(W1[bi]^T X) -- W1 streamed per (bi, mp) tile,
         X (xt) resident; PSUM 3-tag rotation for the ACT/DVE chain.
  GEMM2: y[t, d] = gate_t * (hT^T W2[bi]) -- W2 per block, double-buffered.
W1 tiles for block bi+1 are prefetched on the sync queue BEFORE block
bi's output DMAs are emitted so the first matmuls of bi+1 never wait on
the y-writeback FIFO. Warmup matmuls on a zeroed tile run from t~1us so
the PE HAM clock is at 2.4 GHz when real work starts.
"""

import math

import numpy as np
import ml_dtypes

import concourse.bacc as bacc
import concourse.bass as bass  # noqa: F401
import concourse.mybir as mybir
import concourse.tile as tile
from concourse.bass_utils import run_bass_kernel_spmd
from concourse.tile import add_dep_helper

P = 128
NCORES = 8

f32 = mybir.dt.float32
bf16 = mybir.dt.bfloat16
np_bf16 = ml_dtypes.bfloat16
SILU = mybir.ActivationFunctionType.Silu


def _chunks(tw, step=512):
    out = []
    c0 = 0
    while c0 < tw:
        out.append((c0, min(step, tw - c0)))
        c0 += step
    return out


def build_moe_nc(D, H, TWs, has_b1=False):
    """One SPMD program: len(TWs) token blocks, sizes TWs (multiples of
    128), each block bound to its own W1/W2 slice of the per-core weight
    stream tensors."""
    KO1 = D // P       # GEMM1 contraction tiles (over D)
    MP = H // P        # hidden tiles (per swiGLU half)
    KO2 = H // P       # GEMM2 contraction tiles (over H)
    NB = len(TWs)
    C = sum(TWs)
    TWMAX = max(TWs)
    n2chunks = _chunks(D)

    nc = bacc.Bacc(None)
    xt_d = nc.declare_dram_parameter("xt", [P, KO1, C], bf16, isOutput=False)
    w1_d = nc.declare_dram_parameter(
        "w1", [NB, MP, P, 2, KO1, P], bf16, isOutput=False
    )
    w2_d = nc.declare_dram_parameter("w2", [NB, P, KO2, D], bf16, isOutput=False)
    g_d = nc.declare_dram_parameter("g", [P, C // P], f32, isOutput=False)
    if has_b1:
        b1_d = nc.declare_dram_parameter("b1", [P, NB, 2, MP], f32, isOutput=False)
    y_d = nc.declare_dram_parameter("y", [C, D], f32, isOutput=True)

    W1_PREFETCH = 3   # w1 tiles of block bi+1 pulled ahead of bi's y DMAs

    with tile.TileContext(nc) as tc:
        with (
            tc.tile_pool(name="const", bufs=1) as const,
            tc.tile_pool(name="w1p", bufs=6) as w1p,
            tc.tile_pool(name="w2p", bufs=2) as w2p,
            tc.tile_pool(name="ev", bufs=2) as ev,
            tc.tile_pool(name="ps1", bufs=1, space="PSUM") as ps1,
            tc.tile_pool(name="ps2", bufs=2, space="PSUM") as ps2,
        ):
            # ---- PE warmup: zeroed-bf16 matmuls from t~1us keep the HAM
            # activity window busy so real matmuls start at 2.4 GHz.
            warm = const.tile([P, 640], bf16)
            nc.gpsimd.memset(warm[:], 0.0)
            warm_mms = []
            for wi in range(11):
                wp = ps2.tile([P, 512], f32, tag="psy", name=f"warm{wi}")
                mm = nc.tensor.matmul(
                    wp[:], lhsT=warm[:, :128], rhs=warm[:, 128:640],
                    start=True, stop=True,
                )
                warm_mms.append(mm)

            # ---- resident tensors
            xt_sb = const.tile([P, KO1, C], bf16)
            g_sb = const.tile([P, C // P], f32)
            if has_b1:
                b1_sb = const.tile([P, NB, 2, MP], f32)

            # sync (HWDGE) queue carries ONLY the startup-critical bytes
            # before the first W1 tile: block 0's first token chunk.
            nc.sync.dma_start(xt_sb[:, :, :512], xt_d[:, :, :512])
            # the rest of xt rides SWDGE, anchored behind the first warm
            # matmuls so it doesn't starve the sync queue at t=0.
            xt_rest = []
            r0 = 512
            while r0 < C:
                r1 = min(C, r0 + 896)
                dma = nc.gpsimd.dma_start(xt_sb[:, :, r0:r1], xt_d[:, :, r0:r1])
                xt_rest.append(dma)
                r0 = r1
            for i, dma in enumerate(xt_rest):
                anchor = warm_mms[min(1 + 2 * i, len(warm_mms) - 1)]
                add_dep_helper(dma.ins, anchor.ins, sync=True,
                               reason="stagger xt bulk behind warmup")
            if has_b1:
                nc.gpsimd.dma_start(b1_sb[:], b1_d[:])

            block_mm = {}     # (bi, mp) -> first matmul of that hidden tile

            def _stagger(dma_bi, anchor):
                if anchor is not None:
                    add_dep_helper(dma_bi.ins, anchor.ins, sync=True,
                                   reason="stagger bulk DMA behind compute")

            w1_tiles = {}     # (bi, mp) -> prefetched sbuf tile

            def w1_load(bi, mp):
                t = w1p.tile([P, 2, KO1, P], bf16, tag="w1t",
                             name=f"w1_{bi}_{mp}")
                nc.sync.dma_start(t[:], w1_d[bi, mp])
                return t

            for bi, tw in enumerate(TWs):
                t0 = sum(TWs[:bi])
                hT = ev.tile([P, MP, tw], bf16, tag="hT")
                # ---- GEMM1 + swiGLU ----
                for mp in range(MP):
                    w1t = w1_tiles.pop((bi, mp), None)
                    if w1t is None:
                        w1t = w1_load(bi, mp)
                    psa = ps1.tile([P, TWMAX], f32, tag=f"g1_{(2 * mp) % 3}")
                    psb = ps1.tile([P, TWMAX], f32, tag=f"g1_{(2 * mp + 1) % 3}")
                    for c0, cw in _chunks(tw):
                        for k in range(KO1):
                            mm = nc.tensor.matmul(
                                psa[:, c0:c0 + cw],
                                lhsT=w1t[:, 0, k, :],
                                rhs=xt_sb[:, k, t0 + c0:t0 + c0 + cw],
                                start=(k == 0), stop=(k == KO1 - 1),
                            )
                            block_mm.setdefault((bi, mp), mm)
                        for k in range(KO1):
                            nc.tensor.matmul(
                                psb[:, c0:c0 + cw],
                                lhsT=w1t[:, 1, k, :],
                                rhs=xt_sb[:, k, t0 + c0:t0 + c0 + cw],
                                start=(k == 0), stop=(k == KO1 - 1),
                            )
                    sg = ev.tile([P, TWMAX], f32, tag="sg")
                    if has_b1:
                        nc.scalar.activation(sg[:, :tw], psa[:, :tw], SILU,
                                             bias=b1_sb[:, bi, 0, mp:mp + 1])
                        bs = ev.tile([P, TWMAX], f32, tag="bs")
                        nc.vector.tensor_scalar_add(
                            bs[:, :tw], psb[:, :tw],
                            b1_sb[:, bi, 1, mp:mp + 1])
                        nc.vector.tensor_mul(hT[:, mp, :], sg[:, :tw],
                                             bs[:, :tw])
                    else:
                        nc.scalar.activation(sg[:, :tw], psa[:, :tw], SILU)
                        nc.vector.tensor_mul(hT[:, mp, :], sg[:, :tw],
                                             psb[:, :tw])

                # ---- bulk W2/gate loads for this block ride SWDGE during
                # this block's GEMM1 (needed only at its GEMM2).
                if bi == 0:
                    w2_sb = {}
                if bi < NB:
                    w2_sb[bi] = w2p.tile([P, KO2, D], bf16, tag="w2",
                                         name=f"w2_{bi}")
                    kstep = max(1, KO2 // 4)
                    for ci, k0 in enumerate(range(0, KO2, kstep)):
                        k1 = min(KO2, k0 + kstep)
                        dma = nc.gpsimd.dma_start(
                            w2_sb[bi][:, k0:k1, :], w2_d[bi, :, k0:k1, :]
                        )
                        anchor_mp = min(1 + 3 * ci, MP - 1)
                        _stagger(dma, block_mm.get((bi, anchor_mp)))
                if bi == 0:
                    dma = nc.gpsimd.dma_start(g_sb[:], g_d[:])
                    _stagger(dma, block_mm.get((0, 0)))

                # ---- prefetch next block's first W1 tiles on the sync
                # queue BEFORE this block's y DMAs join that FIFO.
                if bi + 1 < NB:
                    for mp in range(W1_PREFETCH):
                        w1_tiles[(bi + 1, mp)] = w1_load(bi + 1, mp)

                # ---- GEMM2 + gate scale ----
                for mt in range(tw // P):
                    ti = t0 // P + mt
                    rows = slice(t0 + mt * P, t0 + (mt + 1) * P)
                    for n0, nw in n2chunks:
                        psy = ps2.tile([P, 512], f32, tag="psy")
                        for k in range(KO2):
                            nc.tensor.matmul(
                                psy[:, :nw],
                                lhsT=hT[:, k, mt * P:(mt + 1) * P],
                                rhs=w2_sb[bi][:, k, n0:n0 + nw],
                                start=(k == 0), stop=(k == KO2 - 1),
                            )
                        ysb = ev.tile([P, 512], f32, tag="ysb", bufs=3)
                        nc.vector.tensor_scalar_mul(
                            ysb[:, :nw], psy[:, :nw], g_sb[:, ti:ti + 1]
                        )
                        nc.sync.dma_start(y_d[rows, n0:n0 + nw], ysb[:, :nw])
    nc.finalize()
    return nc


def _route(x2, Wr):
    """Top-2 router, numpy fp32 (mirrors jax.lax.top_k + softmax)."""
    n = x2.shape[0]
    ar = np.arange(n)
    z = x2 @ Wr
    idx1 = z.argmax(axis=1)
    v1 = z[ar, idx1]
    z2 = z.copy()
    z2[ar, idx1] = -np.inf
    idx2 = z2.argmax(axis=1)
    v2 = z2[ar, idx2]
    m = np.maximum(v1, v2)
    e1 = np.exp(v1 - m)
    e2 = np.exp(v2 - m)
    s = e1 + e2
    return idx1, idx2, (e1 / s).astype(np.float32), (e2 / s).astype(np.float32)


def _pack_slots(tile_counts, ncores=NCORES):
    """Choose a per-core block pattern (same sizes on every core) and an
    expert label for every (core, block) slot such that each expert's
    128-token tiles are covered by whole slots. Returns (pattern, labels)
    with labels[core][block] = expert id."""
    E = len(tile_counts)
    ntc = max(1, math.ceil(sum(tile_counts) / ncores))
    for _ in range(64):
        # pattern: fives+fours covering ntc tiles per core
        r = ntc % 4
        if ntc >= 5 * r:
            n5, n4 = r, (ntc - 5 * r) // 4
        else:
            n5, n4 = 0, 0  # tiny cores: fall back to one block of ntc
        if n5 + n4 == 0 or ntc < 4:
            pattern = [ntc]
        else:
            pattern = [4] * n4 + [5] * n5
        avail = {sz: pattern.count(sz) * ncores for sz in set(pattern)}
        n5a = avail.get(5, 0)
        n4a = avail.get(4, 0)
        order = sorted(range(E), key=lambda e: -tile_counts[e])
        assign = {e: [] for e in range(E)}
        ok = True
        if len(pattern) == 1:
            # one big block per core: expert must fit in whole cores
            for e in order:
                need = tile_counts[e]
                while need > 0:
                    if avail.get(pattern[0], 0) <= 0:
                        ok = False
                        break
                    avail[pattern[0]] -= 1
                    assign[e].append(pattern[0])
                    need -= pattern[0]
                if not ok:
                    break
        else:
            for e in order:
                need = tile_counts[e]
                if need == 0:
                    continue
                # cover `need` tiles with a fours + b fives, min waste
                best = None
                for b in range(0, n5a + 1):
                    a = max(0, -(-(need - 5 * b) // 4))
                    if a > n4a:
                        continue
                    waste = 4 * a + 5 * b - need
                    if waste < 0:
                        continue
                    key = (waste, a + b)
                    if best is None or key < best[0]:
                        best = (key, a, b)
                if best is None:
                    ok = False
                    break
                _, a, b = best
                n4a -= a
                n5a -= b
                assign[e] = [4] * a + [5] * b
        if ok:
            # distribute labeled slots to (core, block) positions
            by_size = {sz: [] for sz in set(pattern)}
            for e in range(E):
                for s in assign[e]:
                    by_size[s].append(e)
            # pad unused slots with expert 0 (zero tokens -> zero gates)
            for sz in set(pattern):
                total = pattern.count(sz) * ncores
                while len(by_size[sz]) < total:
                    by_size[sz].append(0)
            labels = []
            idx = {sz: 0 for sz in set(pattern)}
            for c in range(ncores):
                row = []
                for sz in pattern:
                    row.append(by_size[sz][idx[sz]])
                    idx[sz] += 1
                labels.append(row)
            return pattern, labels
        ntc += 1
    raise RuntimeError("slot packing failed")


def kernel(x, Wr, W1, b1, W2, b2):
    x = np.asarray(x, dtype=np.float32)
    Wr = np.asarray(Wr, dtype=np.float32)
    W1 = np.asarray(W1, dtype=np.float32)
    b1 = np.asarray(b1, dtype=np.float32)
    W2 = np.asarray(W2, dtype=np.float32)
    b2 = np.asarray(b2, dtype=np.float32)

    Bb, T, D = x.shape
    E, _, H2 = W1.shape
    H = H2 // 2
    N = Bb * T
    KO1 = D // P
    MP = H // P
    KO2 = H // P

    x2 = x.reshape(N, D)
    idx1, idx2, g1, g2 = _route(x2, Wr)

    tok = np.concatenate([np.arange(N), np.arange(N)])
    exp = np.concatenate([idx1, idx2])
    gat = np.concatenate([g1, g2])

    toks_e = [tok[exp == e] for e in range(E)]
    gats_e = [gat[exp == e] for e in range(E)]
    tiles = [math.ceil(len(t) / P) for t in toks_e]

    pattern, labels = _pack_slots(tiles)
    NB = len(pattern)
    TWs = [sz * P for sz in pattern]
    C = sum(TWs)

    # fill each expert's slots with its tokens, in (core, block) order
    slot_fill = {}   # (core, block) -> (token_idx_array, gate_array)
    cursor = [0] * E
    for c in range(NCORES):
        for b in range(NB):
            e = labels[c][b]
            cap = TWs[b]
            lo = cursor[e]
            hi = min(len(toks_e[e]), lo + cap)
            cursor[e] = hi
            slot_fill[(c, b)] = (toks_e[e][lo:hi], gats_e[e][lo:hi])
    for e in range(E):
        assert cursor[e] == len(toks_e[e]), "packing lost tokens"

    has_b1 = bool(np.any(b1))
    nc = build_moe_nc(D, H, TWs, has_b1=has_b1)

    # per-expert transposed weights, cast once
    x2b = x2.astype(np_bf16)
    w1T = [np.ascontiguousarray(
        W1[e].reshape(KO1, P, 2, MP, P).transpose(3, 1, 2, 0, 4)
    ).astype(np_bf16) for e in range(E)]
    w2T = [np.ascontiguousarray(
        W2[e].reshape(KO2, P, D).transpose(1, 0, 2)
    ).astype(np_bf16) for e in range(E)]

    in_maps = []
    for c in range(NCORES):
        xt = np.zeros((C, D), dtype=np_bf16)
        g = np.zeros(C, dtype=np.float32)
        t0 = 0
        for b in range(NB):
            tk, gt = slot_fill[(c, b)]
            xt[t0:t0 + len(tk)] = x2b[tk]
            g[t0:t0 + len(tk)] = gt
            t0 += TWs[b]
        xt_t = np.ascontiguousarray(
            xt.T.reshape(KO1, P, C).transpose(1, 0, 2))
        g_t = np.ascontiguousarray(g.reshape(C // P, P).T)
        w1s = np.stack([w1T[labels[c][b]] for b in range(NB)])
        w2s = np.stack([w2T[labels[c][b]] for b in range(NB)])
        im = {"xt": xt_t, "w1": w1s, "w2": w2s, "g": g_t}
        if has_b1:
            # [P, NB, 2, MP]: partition dim (hidden-within-tile) first
            im["b1"] = np.ascontiguousarray(np.stack(
                [b1[labels[c][b]].reshape(2, MP, P) for b in range(NB)]
            ).transpose(3, 0, 1, 2))
        in_maps.append(im)

    res = run_bass_kernel_spmd(nc, in_maps, list(range(NCORES)))

    out = np.zeros((N, D), dtype=np.float32)
    for c in range(NCORES):
        y = res.results[c]["y"]
        t0 = 0
        for b in range(NB):
            tk, _ = slot_fill[(c, b)]
            if len(tk):
                np.add.at(out, tk, y[t0:t0 + len(tk)])
            t0 += TWs[b]

    if np.any(b2):
        # y_e += b2[e] before gating => out += sum_e comb[:, e] * b2[e]
        comb = np.zeros((N, E), dtype=np.float32)
        comb[np.arange(N), idx1] += g1
        comb[np.arange(N), idx2] += g2
        out += comb @ b2
    return out.reshape(Bb, T, D)
